# revision 31
# baseline (speedup 1.0000x reference)
"""Trainium2 Bass kernel for ExtractRelevantPatches (pool -> top-k -> gather).

Full-input contract: kernel(heatmap [64,448,448,1] f32, image [64,448,448,3] f32)
-> [1344, 64, 64, 3] f32.

Sharding: pure data-parallel over batch; 8 batches per NeuronCore, 8 cores.

Per-core algorithm (raw Bass, explicit semaphores):
  1. DMA heatmap rows (3584 rows x 448 cols) into SBUF, 4 chunks of
     [128 partitions, 7, 448] (row R = 128*(7q+n)+p), on HWDGE (sync).
  2. DVE reduce_sum over 64-column groups -> red [128, 28, 7].
  3. TensorE matmul with 0/1 group matrix G [128,2] (G[p,g]=1 iff p//64==g)
     -> PSUM [2, 196]; PS[g, n*7+bc] = row-group sum for group Gg=2n+g.
  4. Affine DRAM-roundtrip shuffle to V [8, 49] (per-batch pooled sums,
     V[b, br*7+bc]); ranking by sums == ranking by means.
  5. Top-24 via 3 rounds of vector.max + max_index + match_replace;
     keep first 21 indices per batch (descending, as in jax top_k).
  6. base_bk[b,k] = idx + 441*(idx//7)  (patch-row units of 192 elems);
     DRAM roundtrip to a [1,168] f32 row, replicated x4 along free dim and
     broadcast to 16 partitions via a K=1 ones matmul; added to a static
     int16 table (7*j wrap + 3136*b terms) -> dma_gather index list,
     position i=R at idxs[R%16, R//16] (R = global output patch-row).
  7. One dma_gather (InstDMAGatherAnt): 10752 patch-rows (768B each)
     DRAM->SBUF [128, 84, 192], row R at [R%128, R//128].
  8. One store SBUF->DRAM with AP out[(c p) e -> p c e].
"""

import numpy as np

_N_CORES = 8
_B = 64
_B_LOC = _B // _N_CORES  # 8
_PATCH = 64
_GRID = 7
_NPATCH = 21
_PROW = _PATCH * 3            # 192 elements per patch-row
_OUT_ROWS_LOC = _B_LOC * _NPATCH  # 168
_NIDX = _OUT_ROWS_LOC * _PATCH    # 10752 patch-rows per core

_nc_cache = None


def build_program():
    """Build the per-core SPMD Bass program (cached)."""
    global _nc_cache
    if _nc_cache is not None:
        return _nc_cache

    import concourse.bass as bass
    import concourse.bacc as bacc
    import concourse.mybir as mybir

    f32 = mybir.dt.float32
    i16 = mybir.dt.int16
    i32 = mybir.dt.int32
    u32 = mybir.dt.uint32
    X = mybir.AxisListType.X
    Op = mybir.AluOpType

    nc = bacc.Bacc(num_swdge_queues=4)

    hm_in = nc.declare_dram_parameter(
        "heatmap", [_B_LOC, 448, 448, 1], f32, isOutput=False)
    img_in = nc.declare_dram_parameter(
        "image", [_B_LOC, 448, 448, 3], f32, isOutput=False)
    out_t = nc.declare_dram_parameter(
        "out", [_OUT_ROWS_LOC, _PATCH, _PATCH, 3], f32, isOutput=True)

    # DRAM scratch for cross-partition shuffles
    pool_tmp = nc.dram_tensor("pool_tmp", [392], f32)
    base_tmp = nc.dram_tensor("base_tmp", [_OUT_ROWS_LOC], f32)

    # Static part of the gather index list, int16 [16, 672]:
    # position i = R sits at [R%16, R//16]; R = 16*s + w;
    # static term = 7*(R%64) + 3136*(R//1344) = 112*(s%4) + 7*w + 3136*(s//84)
    s_ar = np.arange(672, dtype=np.int64)
    w_ar = np.arange(16, dtype=np.int64)
    st = (112 * (s_ar[None, :] % 4) + 7 * w_ar[:, None]
          + 3136 * (s_ar[None, :] // 84)).astype(np.int16)
    st = np.tile(st, (8, 1))  # replicate across the 8 gpsimd cores
    sttab_const = nc.inline_tensor(st, name="sttab_const")

    # heatmap rows view: [3584, 448] -> [128, 28, 448] (R = 128*m + p)
    hm_tiled = (hm_in[:]
                .rearrange("b r c one -> (b r) (c one)")
                .rearrange("(m p) c -> p m c", p=128))

    # image patch-row view: [25088, 192], offset 0
    img_rows = (img_in[:]
                .rearrange("b r c ch -> (b r c ch)")
                .rearrange("(n e) -> n e", e=_PROW))

    # output patch-row view [10752, 192] -> [p, c, e] with R = 128*c + p
    out_pc = (out_t[:]
              .rearrange("r a b c -> (r a b c)")
              .rearrange("(n e) -> n e", e=_PROW)
              .rearrange("(c p) e -> p c e", p=128))

    from contextlib import ExitStack

    with ExitStack() as ctx:
        e = ctx.enter_context
        chunks = [e(nc.sbuf_tensor(f"hm{i}", [128, 4, 448], f32))
                  for i in range(7)]
        red = e(nc.sbuf_tensor("red", [128, 28, 7], f32))
        G = e(nc.sbuf_tensor("G", [128, 2], f32))
        p2 = e(nc.sbuf_tensor("p2", [2, 196], f32))
        V = e(nc.sbuf_tensor("V", [8, 49], f32))
        vwork = e(nc.sbuf_tensor("vwork", [8, 49], f32))
        m8 = e(nc.sbuf_tensor("m8", [8, 8], f32))
        idx_u = e(nc.sbuf_tensor("idx_u", [8, 24], u32))
        idx_i = e(nc.sbuf_tensor("idx_i", [8, _NPATCH], i32))
        br_i = e(nc.sbuf_tensor("br_i", [8, _NPATCH], i32))
        base_bk = e(nc.sbuf_tensor("base_bk", [8, _NPATCH], i32))
        base_f = e(nc.sbuf_tensor("base_f", [8, _NPATCH], f32))
        basef = e(nc.sbuf_tensor("basef", [1, 168], f32))
        ones128 = e(nc.sbuf_tensor("ones128", [1, 128], f32))
        sttab = e(nc.sbuf_tensor("sttab", [128, 672], i16))
        idx16 = e(nc.sbuf_tensor("idx16", [128, 672], i16))
        GT = e(nc.sbuf_tensor("GT", [128, 84, _PROW], f32))
        ps = e(nc.psum_tensor("ps", [2, 196], f32))
        psD = e(nc.psum_tensor("psD", [128, 672], f32))
        s_load = [e(nc.semaphore(f"s_load{i}")) for i in range(7)]
        s_stt = e(nc.semaphore("s_stt"))
        s_red = e(nc.semaphore("s_red"))
        s_mm = e(nc.semaphore("s_mm"))
        s_ps = e(nc.semaphore("s_ps"))
        s_shuf = e(nc.semaphore("s_shuf"))
        s_topk = e(nc.semaphore("s_topk"))
        s_base = e(nc.semaphore("s_base"))
        s_ones = e(nc.semaphore("s_ones"))
        s_mmD = e(nc.semaphore("s_mmD"))
        s_idx = e(nc.semaphore("s_idx"))
        s_gq = [e(nc.semaphore(f"s_gq{i}")) for i in range(12)]
        s_st = e(nc.semaphore("s_st"))
        block = e(nc.Block())

        @block.sync
        def _(sync):
            sync.dma_start(out=sttab[:], in_=sttab_const[:]).then_inc(s_stt, 16)
            for q in range(7):
                sync.dma_start(
                    out=chunks[q][:],
                    in_=hm_tiled[:, 4 * q:4 * q + 4, :],
                ).then_inc(s_load[q], 16)
            for j in range(4):
                for k in range(3 * j, 3 * j + 3):
                    sync.wait_ge(s_gq[k], 16)
                sync.dma_start(
                    out=out_pc[:, 21 * j:21 * j + 21, :],
                    in_=GT[:, 21 * j:21 * j + 21, :],
                ).then_inc(s_st, 16)
            sync.wait_ge(s_st, 64)

        @block.vector
        def _(vector):
            # constants (disjoint writes, no deps)
            vector.memset(G[0:64, 0:1], 1.0)
            vector.memset(G[0:64, 1:2], 0.0)
            vector.memset(G[64:128, 0:1], 0.0)
            vector.memset(G[64:128, 1:2], 1.0)
            vector.memset(ones128[:], 1.0)
            vector.drain().then_inc(s_ones, 1)
            # column-group reduce per chunk
            for q in range(7):
                vector.wait_ge(s_load[q], 16)
                vector.reduce_sum(
                    out=red[:, 4 * q:4 * q + 4, :],
                    in_=chunks[q][:].rearrange("p n (bc u) -> p n bc u", u=64),
                    axis=X,
                )
            vector.drain().then_inc(s_red, 1)
            # PSUM -> SBUF copy after matmul
            vector.wait_ge(s_mm, 1)
            vector.tensor_copy(out=p2[:], in_=ps[:])
            vector.drain().then_inc(s_ps, 1)
            # top-24
            vector.wait_ge(s_shuf, 32)
            cur = V
            for r3 in range(3):
                vector.max(out=m8[:], in_=cur[:])
                vector.drain()
                vector.max_index(
                    out=idx_u[:, 8 * r3:8 * r3 + 8], in_max=m8[:],
                    in_values=cur[:])
                if r3 < 2:
                    nxt = vwork if r3 == 0 else V
                    vector.match_replace(
                        out=nxt[:], in_to_replace=m8[:], in_values=cur[:],
                        imm_value=-1e30)
                    vector.drain()
                    cur = nxt
            vector.drain()
            # index math: br = idx//7 via (idx*9363)>>16;
            # base = idx + 441*br  (in 192-element patch-row units)
            vector.tensor_copy(out=idx_i[:], in_=idx_u[:, :_NPATCH])
            vector.drain()
            vector.tensor_scalar(
                out=br_i[:], in0=idx_i[:], scalar1=9363, scalar2=None,
                op0=Op.mult)
            vector.drain()
            vector.tensor_scalar(
                out=br_i[:], in0=br_i[:], scalar1=16, scalar2=None,
                op0=Op.logical_shift_right)
            vector.drain()
            vector.tensor_scalar(
                out=br_i[:], in0=br_i[:], scalar1=441, scalar2=None,
                op0=Op.mult)
            vector.drain()
            vector.tensor_tensor(
                out=base_bk[:], in0=idx_i[:], in1=br_i[:],
                op=Op.add)
            vector.drain()
            vector.tensor_copy(out=base_f[:], in_=base_bk[:])
            vector.drain().then_inc(s_topk, 1)
            # idx16[0:16] = cast(psD) + sttab
            vector.wait_ge(s_mmD, 2)
            vector.wait_ge(s_stt, 16)
            vector.tensor_copy(out=idx16[:, :], in_=psD[:])
            vector.drain()
            vector.tensor_tensor(
                out=idx16[:, :], in0=idx16[:, :], in1=sttab[:],
                op=Op.add)
            vector.drain().then_inc(s_idx, 1)

        @block.scalar
        def _(sc):
            # small critical-path shuffles on the ACT HWDGE ring (low latency)
            # shuffle [2,196] -> [8,49] via DRAM:
            # PS[g2, (t*7+s2)*7+bc] is group Gg=14t+2*s2+g2 -> DRAM pos
            # D = b*49+br*7+bc = 98t+14s2+7*g2+bc (affine).
            sc.wait_ge(s_ps, 1)
            sc.dma_start(
                out=pool_tmp[:].rearrange("(t s2 x bc) -> x t s2 bc",
                                          t=4, s2=7, x=2),
                in_=p2[:].rearrange("x (t s2 bc) -> x t s2 bc", t=4, s2=7),
            ).then_inc(s_shuf, 16)
            sc.wait_ge(s_shuf, 16)
            sc.dma_start(
                out=V[:],
                in_=pool_tmp[:].rearrange("(b w) -> b w", b=8)[:, :49],
            ).then_inc(s_shuf, 16)
            # base spread [8,21] -> [168] -> one f32 row
            sc.wait_ge(s_topk, 1)
            sc.dma_start(
                out=base_tmp[:].rearrange("(b k) -> b k", b=8),
                in_=base_f[:],
            ).then_inc(s_base, 16)
            sc.wait_ge(s_base, 16)
            sc.dma_start(
                out=basef[:],
                in_=base_tmp[:].rearrange("(one m) -> one m", one=1),
            ).then_inc(s_base, 16)

        @block.tensor
        def _(tensor):
            tensor.wait_ge(s_red, 1)
            tensor.matmul(
                out=ps[:],
                lhsT=G[:],
                rhs=red[:].rearrange("p n bc -> p (n bc)"),
                start=True,
                stop=True,
            ).then_inc(s_mm, 1)
            # replicate base row to 16 partitions, x4 along free dim:
            # psD[w, s] = basef[0, s//4]
            tensor.wait_ge(s_base, 32)
            tensor.wait_ge(s_ones, 1)
            bb = basef[:1, :].rearrange("p (m one) -> p m one", one=1)
            tensor.matmul(
                out=psD[:, 0:512],
                lhsT=ones128[:],
                rhs=bb[:, 0:128, :].to_broadcast([1, 128, 4]),
                start=True, stop=True,
            ).then_inc(s_mmD, 1)
            tensor.matmul(
                out=psD[:, 512:672],
                lhsT=ones128[:],
                rhs=bb[:, 128:168, :].to_broadcast([1, 40, 4]),
                start=True, stop=True,
            ).then_inc(s_mmD, 1)

        @block.gpsimd
        def _(g):
            # preload the extended-instruction library early so the ucode
            # overlay DMA overlaps the heatmap phase
            from concourse import library_config
            g.load_library(library_config.mlp)
            # the gather: 10752 patch-rows of 192 f32, row R -> [R%128, R//128]
            # 12 chunks of 896 idxs (SWDGE ring holds ~64-96 descs/DMA)
            g.wait_ge(s_idx, 1)
            for k in range(12):
                g.dma_gather(
                    out_ap=GT[:, 7 * k:7 * k + 7, :],
                    in_ap=img_rows,
                    idxs_ap=idx16[:, 56 * k:56 * k + 56],
                    num_idxs=896,
                    num_idxs_reg=896,
                    elem_size=_PROW,
                    queue_num=1 + (k % 3),
                ).then_inc(s_gq[k], 16)

    nc.finalize()
    _nc_cache = nc
    return nc


def kernel(heatmap, image):
    from concourse.bass_utils import run_bass_kernel_spmd

    heatmap = np.ascontiguousarray(np.asarray(heatmap), dtype=np.float32)
    image = np.ascontiguousarray(np.asarray(image), dtype=np.float32)
    assert heatmap.shape == (_B, 448, 448, 1)
    assert image.shape == (_B, 448, 448, 3)

    nc = build_program()
    in_maps = [
        {
            "heatmap": heatmap[c * _B_LOC:(c + 1) * _B_LOC],
            "image": image[c * _B_LOC:(c + 1) * _B_LOC],
        }
        for c in range(_N_CORES)
    ]
    res = run_bass_kernel_spmd(nc, in_maps, list(range(_N_CORES)))
    outs = [res.results[c]["out"] for c in range(_N_CORES)]
    return np.concatenate(outs, axis=0)


# revision 32
# speedup vs baseline: 1.0460x; 1.0460x over previous
"""Trainium2 Bass kernel for ExtractRelevantPatches (pool -> top-k -> gather).

Full-input contract: kernel(heatmap [64,448,448,1] f32, image [64,448,448,3] f32)
-> [1344, 64, 64, 3] f32.

Sharding: pure data-parallel over batch; 8 batches per NeuronCore, 8 cores.

Per-core algorithm (raw Bass, explicit semaphores):
  1. DMA heatmap rows (3584 rows x 448 cols) into SBUF, 4 chunks of
     [128 partitions, 7, 448] (row R = 128*(7q+n)+p), on HWDGE (sync).
  2. DVE reduce_sum over 64-column groups -> red [128, 28, 7].
  3. TensorE matmul with 0/1 group matrix G [128,2] (G[p,g]=1 iff p//64==g)
     -> PSUM [2, 196]; PS[g, n*7+bc] = row-group sum for group Gg=2n+g.
  4. Affine DRAM-roundtrip shuffle to V [8, 49] (per-batch pooled sums,
     V[b, br*7+bc]); ranking by sums == ranking by means.
  5. Top-24 via 3 rounds of vector.max + max_index + match_replace;
     keep first 21 indices per batch (descending, as in jax top_k).
  6. base_bk[b,k] = idx + 441*(idx//7)  (patch-row units of 192 elems);
     DRAM roundtrip to a [1,168] f32 row, replicated x4 along free dim and
     broadcast to 16 partitions via a K=1 ones matmul; added to a static
     int16 table (7*j wrap + 3136*b terms) -> dma_gather index list,
     position i=R at idxs[R%16, R//16] (R = global output patch-row).
  7. One dma_gather (InstDMAGatherAnt): 10752 patch-rows (768B each)
     DRAM->SBUF [128, 84, 192], row R at [R%128, R//128].
  8. One store SBUF->DRAM with AP out[(c p) e -> p c e].
"""

import numpy as np

_N_CORES = 8
_B = 64
_B_LOC = _B // _N_CORES  # 8
_PATCH = 64
_GRID = 7
_NPATCH = 21
_PROW = _PATCH * 3            # 192 elements per patch-row
_OUT_ROWS_LOC = _B_LOC * _NPATCH  # 168
_NIDX = _OUT_ROWS_LOC * _PATCH    # 10752 patch-rows per core

_nc_cache = None


def build_program():
    """Build the per-core SPMD Bass program (cached)."""
    global _nc_cache
    if _nc_cache is not None:
        return _nc_cache

    import concourse.bass as bass
    import concourse.bacc as bacc
    import concourse.mybir as mybir

    f32 = mybir.dt.float32
    i16 = mybir.dt.int16
    i32 = mybir.dt.int32
    u32 = mybir.dt.uint32
    X = mybir.AxisListType.X
    Op = mybir.AluOpType

    nc = bacc.Bacc(num_swdge_queues=4)

    hm_in = nc.declare_dram_parameter(
        "heatmap", [_B_LOC, 448, 448, 1], f32, isOutput=False)
    img_in = nc.declare_dram_parameter(
        "image", [_B_LOC, 448, 448, 3], f32, isOutput=False)
    out_t = nc.declare_dram_parameter(
        "out", [_OUT_ROWS_LOC, _PATCH, _PATCH, 3], f32, isOutput=True)

    # DRAM scratch for cross-partition shuffles
    pool_tmp = nc.dram_tensor("pool_tmp", [392], f32)
    base_tmp = nc.dram_tensor("base_tmp", [_OUT_ROWS_LOC], f32)

    # Static part of the gather index list, int16 [16, 672]:
    # position i = R sits at [R%16, R//16]; R = 16*s + w;
    # static term = 7*(R%64) + 3136*(R//1344) = 112*(s%4) + 7*w + 3136*(s//84)
    s_ar = np.arange(672, dtype=np.int64)
    w_ar = np.arange(16, dtype=np.int64)
    st = (112 * (s_ar[None, :] % 4) + 7 * w_ar[:, None]
          + 3136 * (s_ar[None, :] // 84)).astype(np.int16)
    st = np.tile(st, (8, 1))  # replicate across the 8 gpsimd cores
    sttab_const = nc.inline_tensor(st, name="sttab_const")

    # heatmap rows view: [3584, 448] -> [128, 28, 448] (R = 128*m + p)
    hm_tiled = (hm_in[:]
                .rearrange("b r c one -> (b r) (c one)")
                .rearrange("(m p) c -> p m c", p=128))

    # image patch-row view: [25088, 192], offset 0
    img_rows = (img_in[:]
                .rearrange("b r c ch -> (b r c ch)")
                .rearrange("(n e) -> n e", e=_PROW))

    # output patch-row view [10752, 192] -> [p, c, e] with R = 128*c + p
    out_pc = (out_t[:]
              .rearrange("r a b c -> (r a b c)")
              .rearrange("(n e) -> n e", e=_PROW)
              .rearrange("(c p) e -> p c e", p=128))

    from contextlib import ExitStack

    with ExitStack() as ctx:
        e = ctx.enter_context
        chunks = [e(nc.sbuf_tensor(f"hm{i}", [128, 4, 448], f32))
                  for i in range(7)]
        red = e(nc.sbuf_tensor("red", [128, 28, 7], f32))
        G = e(nc.sbuf_tensor("G", [128, 2], f32))
        p2 = e(nc.sbuf_tensor("p2", [2, 196], f32))
        V = e(nc.sbuf_tensor("V", [8, 49], f32))
        vwork = e(nc.sbuf_tensor("vwork", [8, 49], f32))
        m8 = e(nc.sbuf_tensor("m8", [8, 8], f32))
        idx_u = e(nc.sbuf_tensor("idx_u", [8, 24], u32))
        idx_i = e(nc.sbuf_tensor("idx_i", [8, _NPATCH], i32))
        br_i = e(nc.sbuf_tensor("br_i", [8, _NPATCH], i32))
        base_bk = e(nc.sbuf_tensor("base_bk", [8, _NPATCH], i32))
        base_f = e(nc.sbuf_tensor("base_f", [8, _NPATCH], f32))
        basef = e(nc.sbuf_tensor("basef", [1, 168], f32))
        ones128 = e(nc.sbuf_tensor("ones128", [1, 128], f32))
        sttab = e(nc.sbuf_tensor("sttab", [128, 672], i16))
        idx16 = e(nc.sbuf_tensor("idx16", [128, 672], i16))
        GT = e(nc.sbuf_tensor("GT", [128, 84, _PROW], f32))
        ps = e(nc.psum_tensor("ps", [2, 196], f32))
        psD = e(nc.psum_tensor("psD", [128, 672], f32))
        s_load = [e(nc.semaphore(f"s_load{i}")) for i in range(7)]
        s_stt = e(nc.semaphore("s_stt"))
        s_red = e(nc.semaphore("s_red"))
        s_mm = e(nc.semaphore("s_mm"))
        s_ps = e(nc.semaphore("s_ps"))
        s_shuf = e(nc.semaphore("s_shuf"))
        s_topk = e(nc.semaphore("s_topk"))
        s_base = e(nc.semaphore("s_base"))
        s_ones = e(nc.semaphore("s_ones"))
        s_mmD = e(nc.semaphore("s_mmD"))
        s_idx = e(nc.semaphore("s_idx"))
        s_gq = [e(nc.semaphore(f"s_gq{i}")) for i in range(12)]
        s_st = e(nc.semaphore("s_st"))
        block = e(nc.Block())

        @block.sync
        def _(sync):
            sync.dma_start(out=sttab[:], in_=sttab_const[:]).then_inc(s_stt, 16)
            for q in range(7):
                sync.dma_start(
                    out=chunks[q][:],
                    in_=hm_tiled[:, 4 * q:4 * q + 4, :],
                ).then_inc(s_load[q], 16)
            for j in range(4):
                for k in range(3 * j, 3 * j + 3):
                    sync.wait_ge(s_gq[k], 16)
                sync.dma_start(
                    out=out_pc[:, 21 * j:21 * j + 21, :],
                    in_=GT[:, 21 * j:21 * j + 21, :],
                ).then_inc(s_st, 16)
            sync.wait_ge(s_st, 64)

        @block.vector
        def _(vector):
            # constants (disjoint writes, no deps)
            vector.memset(G[0:64, 0:1], 1.0)
            vector.memset(G[0:64, 1:2], 0.0)
            vector.memset(G[64:128, 0:1], 0.0)
            vector.memset(G[64:128, 1:2], 1.0)
            vector.memset(ones128[:], 1.0)
            vector.drain().then_inc(s_ones, 1)
            # column-group reduce per chunk
            for q in range(7):
                vector.wait_ge(s_load[q], 16)
                vector.reduce_sum(
                    out=red[:, 4 * q:4 * q + 4, :],
                    in_=chunks[q][:].rearrange("p n (bc u) -> p n bc u", u=64),
                    axis=X,
                )
            vector.drain().then_inc(s_red, 1)
            # PSUM -> SBUF copy after matmul
            vector.wait_ge(s_mm, 1)
            vector.tensor_copy(out=p2[:], in_=ps[:])
            vector.drain().then_inc(s_ps, 1)
            # top-24
            vector.wait_ge(s_shuf, 32)
            cur = V
            for r3 in range(3):
                vector.max(out=m8[:], in_=cur[:])
                vector.drain()
                vector.max_index(
                    out=idx_u[:, 8 * r3:8 * r3 + 8], in_max=m8[:],
                    in_values=cur[:])
                if r3 < 2:
                    nxt = vwork if r3 == 0 else V
                    vector.match_replace(
                        out=nxt[:], in_to_replace=m8[:], in_values=cur[:],
                        imm_value=-1e30)
                    vector.drain()
                    cur = nxt
            vector.drain()
            # index math: br = idx//7 via (idx*9363)>>16;
            # base = idx + 441*br  (in 192-element patch-row units)
            vector.tensor_copy(out=idx_i[:], in_=idx_u[:, :_NPATCH])
            vector.drain()
            vector.tensor_scalar(
                out=br_i[:], in0=idx_i[:], scalar1=9363, scalar2=None,
                op0=Op.mult)
            vector.drain()
            vector.tensor_scalar(
                out=br_i[:], in0=br_i[:], scalar1=16, scalar2=None,
                op0=Op.logical_shift_right)
            vector.drain()
            vector.tensor_scalar(
                out=br_i[:], in0=br_i[:], scalar1=441, scalar2=None,
                op0=Op.mult)
            vector.drain()
            vector.tensor_tensor(
                out=base_bk[:], in0=idx_i[:], in1=br_i[:],
                op=Op.add)
            vector.drain()
            vector.tensor_copy(out=base_f[:], in_=base_bk[:])
            vector.drain().then_inc(s_topk, 1)
            # idx16[0:16] = cast(psD) + sttab
            vector.wait_ge(s_mmD, 2)
            vector.wait_ge(s_stt, 16)
            vector.tensor_copy(out=idx16[:, :], in_=psD[:])
            vector.drain()
            vector.tensor_tensor(
                out=idx16[:, :], in0=idx16[:, :], in1=sttab[:],
                op=Op.add)
            vector.drain().then_inc(s_idx, 1)

        @block.scalar
        def _(sc):
            # small critical-path shuffles on the ACT HWDGE ring (low latency)
            # shuffle [2,196] -> [8,49] via DRAM:
            # PS[g2, (t*7+s2)*7+bc] is group Gg=14t+2*s2+g2 -> DRAM pos
            # D = b*49+br*7+bc = 98t+14s2+7*g2+bc (affine).
            sc.wait_ge(s_ps, 1)
            sc.dma_start(
                out=pool_tmp[:].rearrange("(t s2 x bc) -> x t s2 bc",
                                          t=4, s2=7, x=2),
                in_=p2[:].rearrange("x (t s2 bc) -> x t s2 bc", t=4, s2=7),
            ).then_inc(s_shuf, 16)
            sc.wait_ge(s_shuf, 16)
            sc.dma_start(
                out=V[:],
                in_=pool_tmp[:].rearrange("(b w) -> b w", b=8)[:, :49],
            ).then_inc(s_shuf, 16)
            # base spread [8,21] -> [168] -> one f32 row
            sc.wait_ge(s_topk, 1)
            sc.dma_start(
                out=base_tmp[:].rearrange("(b k) -> b k", b=8),
                in_=base_f[:],
            ).then_inc(s_base, 16)
            sc.wait_ge(s_base, 16)
            sc.dma_start(
                out=basef[:],
                in_=base_tmp[:].rearrange("(one m) -> one m", one=1),
            ).then_inc(s_base, 16)

        @block.tensor
        def _(tensor):
            tensor.wait_ge(s_red, 1)
            tensor.matmul(
                out=ps[:],
                lhsT=G[:],
                rhs=red[:].rearrange("p n bc -> p (n bc)"),
                start=True,
                stop=True,
            ).then_inc(s_mm, 1)
            # replicate base row to 16 partitions, x4 along free dim:
            # psD[w, s] = basef[0, s//4]
            tensor.wait_ge(s_base, 32)
            tensor.wait_ge(s_ones, 1)
            bb = basef[:1, :].rearrange("p (m one) -> p m one", one=1)
            tensor.matmul(
                out=psD[:, 0:512],
                lhsT=ones128[:],
                rhs=bb[:, 0:128, :].to_broadcast([1, 128, 4]),
                start=True, stop=True,
            ).then_inc(s_mmD, 1)
            tensor.matmul(
                out=psD[:, 512:672],
                lhsT=ones128[:],
                rhs=bb[:, 128:168, :].to_broadcast([1, 40, 4]),
                start=True, stop=True,
            ).then_inc(s_mmD, 1)

        @block.gpsimd
        def _(g):
            # preload the extended-instruction library early so the ucode
            # overlay DMA overlaps the heatmap phase
            from concourse import library_config
            g.load_library(library_config.mlp)
            # the gather: 10752 patch-rows of 192 f32, row R -> [R%128, R//128]
            # 12 chunks of 896 idxs (SWDGE ring holds ~64-96 descs/DMA)
            g.wait_ge(s_idx, 1)
            for k in range(12):
                g.dma_gather(
                    out_ap=GT[:, 7 * k:7 * k + 7, :],
                    in_ap=img_rows,
                    idxs_ap=idx16[:, 56 * k:56 * k + 56],
                    num_idxs=896,
                    num_idxs_reg=896,
                    elem_size=_PROW,
                    queue_num=k % 4,
                ).then_inc(s_gq[k], 16)

    nc.finalize()
    _nc_cache = nc
    return nc


def kernel(heatmap, image):
    from concourse.bass_utils import run_bass_kernel_spmd

    heatmap = np.ascontiguousarray(np.asarray(heatmap), dtype=np.float32)
    image = np.ascontiguousarray(np.asarray(image), dtype=np.float32)
    assert heatmap.shape == (_B, 448, 448, 1)
    assert image.shape == (_B, 448, 448, 3)

    nc = build_program()
    in_maps = [
        {
            "heatmap": heatmap[c * _B_LOC:(c + 1) * _B_LOC],
            "image": image[c * _B_LOC:(c + 1) * _B_LOC],
        }
        for c in range(_N_CORES)
    ]
    res = run_bass_kernel_spmd(nc, in_maps, list(range(_N_CORES)))
    outs = [res.results[c]["out"] for c in range(_N_CORES)]
    return np.concatenate(outs, axis=0)
